# revision 1
# baseline (speedup 1.0000x reference)
"""Megatron-style tensor-parallel causal attention (BitLinear qkv/o) on 8 TRN2 cores.

Sharding: each core owns 2 of 16 heads (qkv_weight rows) and the matching
256 o_weight columns. x/rotary replicated; partial outputs summed on host.
Matmuls run as float32r (FP22 truncated fp32) at full PE rate with fp32 PSUM
accumulation. Quantized weights are small integers => exact in FP22.
"""

import math

import numpy as np

EPS = 1e-5
NUM_HEADS = 16
HEAD_DIM = 128
B, S, H = 2, 2048, 2048
NCORES = 8
HPC = NUM_HEADS // NCORES        # heads per core = 2
FPC = 3 * HPC * HEAD_DIM         # qkv features per core = 768
P = 128
NHT = H // P                     # 16 h_in tiles
CH = 256                         # proj token chunk
NCH = S // CH                    # 8 chunks per batch
QC = 512                         # attention q chunk
NQC = S // QC                    # 4


def _build_program():
    import concourse.bacc as bacc
    import concourse.mybir as mybir
    import concourse.tile as tile

    f32 = mybir.dt.float32
    f32r = mybir.dt.float32r
    AF = mybir.ActivationFunctionType

    nc = bacc.Bacc(None, target_bir_lowering=False)

    xt = nc.dram_tensor("xt", [B, H, S], f32, kind="ExternalInput")
    wqkv = nc.dram_tensor("wqkv", [H, FPC], f32, kind="ExternalInput")
    wo = nc.dram_tensor("wo", [HPC * HEAD_DIM, H], f32, kind="ExternalInput")
    cos_t = nc.dram_tensor("cos_t", [P, S], f32, kind="ExternalInput")
    sin_s = nc.dram_tensor("sin_s", [P, S], f32, kind="ExternalInput")
    masks = nc.dram_tensor("masks", [P, 4 * QC + P], f32, kind="ExternalInput")
    out = nc.dram_tensor("out", [B, S, H], f32, kind="ExternalOutput")

    def r(ap):
        return ap.bitcast(f32r)

    with tile.TileContext(nc) as tc:
        with tc.tile_pool(name="const", bufs=1) as cpool:
            w_sb = cpool.tile([P, NHT, FPC], f32r)
            nc.sync.dma_start(w_sb[:], wqkv.rearrange("(t p) f -> p t f", p=P).bitcast(f32r))
            wo_sb = cpool.tile([P, 2, H], f32r)
            nc.sync.dma_start(wo_sb[:], wo.rearrange("(t p) o -> p t o", p=P).bitcast(f32r))
            masks_sb = cpool.tile([P, 4 * QC + P], f32r)
            nc.sync.dma_start(masks_sb[:], masks[:].bitcast(f32r))

            rot_sb = cpool.tile([P, 2 * S], f32r)
            nc.sync.dma_start(rot_sb[:, 0:S], cos_t[:].bitcast(f32r))
            nc.sync.dma_start(rot_sb[:, S : 2 * S], sin_s[:].bitcast(f32r))

            for b in range(B):
                with tc.tile_pool(name=f"bat{b}", bufs=1) as bpool:
                    # qk[f]: roped q/k tiles [d, s]; f = (q0,q1,k0,k1)
                    qk = [bpool.tile([P, S], f32r, tag=f"qk{f}", name=f"qk{f}_{b}") for f in range(4)]
                    v_sb = bpool.tile([P, S * HPC], f32r, tag="v")

                    with (
                        tc.tile_pool(name=f"proj{b}", bufs=2) as ppool,
                        tc.psum_pool(name=f"pps{b}", bufs=4) as pps,
                    ):
                        for tcn in range(NCH):
                            xt_sb = ppool.tile([P, NHT, CH], f32r, tag="xt")
                            nc.sync.dma_start(
                                xt_sb[:],
                                xt[b, :, tcn * CH : (tcn + 1) * CH].rearrange(
                                    "(t p) c -> p t c", p=P
                                ).bitcast(f32r),
                            )
                            for f in range(4):
                                ps = pps.tile([P, CH], f32, tag="qk")
                                for h in range(NHT):
                                    nc.tensor.matmul(
                                        ps[:],
                                        lhsT=(w_sb[:, h, f * P : (f + 1) * P]),
                                        rhs=(xt_sb[:, h, :]),
                                        start=(h == 0),
                                        stop=(h == NHT - 1),
                                    )
                                nc.any.tensor_copy(
                                    qk[f][:, tcn * CH : (tcn + 1) * CH], ps[:]
                                )
                            for tsub in range(2):
                                psv = pps.tile([P, 2 * P], f32, tag="v")
                                for h in range(NHT):
                                    nc.tensor.matmul(
                                        psv[:],
                                        lhsT=(xt_sb[:, h, tsub * P : (tsub + 1) * P]),
                                        rhs=(w_sb[:, h, 4 * P : 6 * P]),
                                        start=(h == 0),
                                        stop=(h == NHT - 1),
                                    )
                                kb = 2 * tcn + tsub
                                nc.any.tensor_copy(
                                    v_sb[:, kb * 2 * P : (kb + 1) * 2 * P], psv[:]
                                )
                        # RoPE in place on q/k tiles
                        for f in range(4):
                            m1 = ppool.tile([P, S], f32r, tag="m1", bufs=1)
                            qsw = ppool.tile([P, S], f32r, tag="qsw", bufs=1)
                            tmp = ppool.tile([P, S], f32r, tag="tmp", bufs=1)
                            nc.sync.dma_start(qsw[0:64, :], qk[f][64:128, :])
                            nc.sync.dma_start(qsw[64:128, :], qk[f][0:64, :])
                            nc.vector.tensor_mul(m1[:], qk[f][:], rot_sb[:, 0:S])
                            nc.vector.tensor_mul(tmp[:], qsw[:], rot_sb[:, S : 2 * S])
                            nc.vector.tensor_add(qk[f][:], m1[:], tmp[:])

                    with (
                        tc.tile_pool(name=f"attn{b}", bufs=2) as apool,
                        tc.psum_pool(name=f"aps{b}", bufs=1) as aps,
                    ):
                        for qc in range(NQC):
                            kmax = 4 * qc + 4  # causal k-tile count
                            yn = []
                            for hl in range(2):
                                yt_ps = aps.tile([P, QC], f32, tag="yt")
                                sum_ps = aps.tile([P, QC], f32, tag="sum")
                                for g in range(0, kmax, 2):
                                    sc_ps = aps.tile([P, 2 * QC], f32, tag="sc", bufs=2)
                                    for j2 in range(2):
                                        kb = g + j2
                                        nc.tensor.matmul(
                                            sc_ps[:, j2 * QC : (j2 + 1) * QC],
                                            lhsT=(qk[2 + hl][:, kb * P : (kb + 1) * P]),
                                            rhs=(qk[hl][:, qc * QC : (qc + 1) * QC]),
                                            start=True,
                                            stop=True,
                                        )
                                    ex = apool.tile([P, 2 * QC], f32r, tag="ex", bufs=3)
                                    nc.scalar.activation(ex[:], sc_ps[:], AF.Exp)
                                    for j2 in range(2):
                                        kb = g + j2
                                        if kb >= 4 * qc:
                                            jj = kb - 4 * qc
                                            nc.gpsimd.tensor_mul(
                                                ex[:, j2 * QC : (j2 + 1) * QC],
                                                ex[:, j2 * QC : (j2 + 1) * QC],
                                                masks_sb[:, jj * QC : (jj + 1) * QC],
                                            )
                                    for j2 in range(2):
                                        kb = g + j2
                                        nc.tensor.matmul(
                                            yt_ps[:],
                                            lhsT=(v_sb[:, kb * 2 * P + hl * P : kb * 2 * P + (hl + 1) * P]),
                                            rhs=(ex[:, j2 * QC : (j2 + 1) * QC]),
                                            start=(kb == 0),
                                            stop=(kb == kmax - 1),
                                        )
                                        nc.tensor.matmul(
                                            sum_ps[:],
                                            lhsT=(masks_sb[:, 4 * QC : 4 * QC + P]),
                                            rhs=(ex[:, j2 * QC : (j2 + 1) * QC]),
                                            start=(kb == 0),
                                            stop=(kb == kmax - 1),
                                        )
                                recip = apool.tile([P, QC], f32, tag="rc")
                                nc.vector.reciprocal(recip[:], sum_ps[:])
                                y = apool.tile([P, QC], f32r, tag=f"yn{hl}")
                                nc.vector.tensor_mul(y[:], yt_ps[:], recip[:])
                                yn.append(y)
                            for tt in range(4):
                                for oc in range(4):
                                    ops = aps.tile([P, QC], f32, tag="op", bufs=2)
                                    for hl in range(2):
                                        nc.tensor.matmul(
                                            ops[:],
                                            lhsT=(yn[hl][:, tt * P : (tt + 1) * P]),
                                            rhs=(wo_sb[:, hl, oc * QC : (oc + 1) * QC]),
                                            start=(hl == 0),
                                            stop=(hl == 1),
                                        )
                                    os_sb = apool.tile([P, QC], f32, tag="os", bufs=4)
                                    if (tt + oc) % 2 == 0:
                                        nc.vector.tensor_copy(os_sb[:], ops[:])
                                    else:
                                        nc.scalar.copy(os_sb[:], ops[:])
                                    nc.sync.dma_start(
                                        out[
                                            b,
                                            qc * QC + tt * P : qc * QC + (tt + 1) * P,
                                            oc * QC : (oc + 1) * QC,
                                        ],
                                        os_sb[:],
                                    )
    nc.finalize()
    return nc


_NC_CACHE = None


def _get_program():
    global _NC_CACHE
    if _NC_CACHE is None:
        _NC_CACHE = _build_program()
    return _NC_CACHE


def kernel(x, rotary, qkv_weight, o_weight):
    import jax
    import jax.numpy as jnp
    from concourse.bass_utils import run_bass_kernel_spmd

    cpu = jax.devices("cpu")[0]
    with jax.default_device(cpu):
        sq = jnp.mean(jnp.abs(jnp.asarray(qkv_weight)))
        wq_q = np.asarray(jnp.round(jnp.asarray(qkv_weight) / (sq + EPS)), np.float32)
        so = jnp.mean(jnp.abs(jnp.asarray(o_weight)))
        wo_q = np.asarray(jnp.round(jnp.asarray(o_weight) / (so + EPS)), np.float32)
        sq = float(sq)
        so = float(so)

    xt = np.ascontiguousarray(x.transpose(0, 2, 1)).astype(np.float32)
    cos_t = np.ascontiguousarray(rotary[1].T).astype(np.float32)
    sin_t = np.ascontiguousarray(rotary[0].T).astype(np.float32)
    sin_s = sin_t.copy()
    sin_s[:64] *= -1.0

    mask = np.zeros((P, 4 * QC + P), np.float32)
    kk = np.arange(P)[:, None]
    qq = np.arange(QC)[None, :]
    for j in range(4):
        mask[:, j * QC : (j + 1) * QC] = (qq >= j * P + kk).astype(np.float32)
    mask[:, 4 * QC :] = 1.0

    sm_scale = np.float32(sq * sq / math.sqrt(HEAD_DIM))
    final_scale = sq * so

    in_maps = []
    for c in range(NCORES):
        # feature order per core: q_h0, q_h1, k_h0, k_h1, v_h0, v_h1 (128 each)
        # softmax scale is folded into the q rows (scores = (q*sm)·k).
        rows = []
        for part in range(3):  # q, k, v blocks of qkv_weight
            for hl in range(HPC):
                g = 2 * c + hl
                blk = wq_q[part * H + g * HEAD_DIM : part * H + (g + 1) * HEAD_DIM]
                if part == 0:
                    blk = blk * sm_scale
                rows.append(blk)
        wqkv_c = np.ascontiguousarray(np.concatenate(rows, axis=0).T).astype(np.float32)  # [H, 768]
        wo_c = np.ascontiguousarray(
            (wo_q[:, c * FPC // 3 : (c + 1) * FPC // 3].T * final_scale).astype(
                np.float32
            )
        )  # [256, H]
        in_maps.append(
            {
                "xt": xt,
                "wqkv": wqkv_c,
                "wo": wo_c,
                "cos_t": cos_t,
                "sin_s": sin_s,
                "masks": mask,
            }
        )

    nc = _get_program()
    res = run_bass_kernel_spmd(nc, in_maps, core_ids=list(range(NCORES)))
    acc = res.results[0]["out"].astype(np.float32)
    for c in range(1, NCORES):
        acc = acc + res.results[c]["out"]
    return acc



# revision 5
# speedup vs baseline: 1.3298x; 1.3298x over previous
"""Megatron-style tensor-parallel causal attention (BitLinear qkv/o) on 8 TRN2 cores.

Sharding: each core owns 2 of 16 heads (qkv_weight rows) and the matching
256 o_weight columns. x/rotary replicated; partial outputs summed on host.

All SBUF data is bf16 (halves DMA + enables DVE 2x modes); PSUM stays fp32.
Causal mask is folded into the score PSUM via an identity-lhsT matmul adding
-60 to masked entries before exp. Softmax denominator uses the all-ones
lhsT matmul (broadcast rows), normalization on DVE before the out-proj.
Emission order proj(b0), proj(b1), attn(b0), attn(b1) keeps the PE busy:
RoPE(b0) on DVE overlaps proj(b1) matmuls, attention overlaps nothing it
needs.
"""

import math

import numpy as np

EPS = 1e-5
NUM_HEADS = 16
HEAD_DIM = 128
B, S, H = 2, 2048, 2048
NCORES = 8
HPC = NUM_HEADS // NCORES        # heads per core = 2
FPC = 3 * HPC * HEAD_DIM         # qkv features per core = 768
P = 128
NHT = H // P                     # 16 h_in tiles
CH = 512                         # proj token chunk
NCH = S // CH                    # 4 chunks per batch
QC = 256                         # attention q chunk
NQC = S // QC                    # 8
MASKV = -60.0


def _build_program():
    import concourse.bacc as bacc
    import concourse.mybir as mybir
    import concourse.tile as tile

    f32 = mybir.dt.float32
    bf16 = mybir.dt.bfloat16
    AF = mybir.ActivationFunctionType

    nc = bacc.Bacc(None, target_bir_lowering=False)

    xt = nc.dram_tensor("xt", [B, H, S], bf16, kind="ExternalInput")
    wqkv = nc.dram_tensor("wqkv", [H, FPC], bf16, kind="ExternalInput")
    wo = nc.dram_tensor("wo", [HPC * HEAD_DIM, H], bf16, kind="ExternalInput")
    cos_t = nc.dram_tensor("cos_t", [P, S], bf16, kind="ExternalInput")
    sin_s = nc.dram_tensor("sin_s", [P, S], bf16, kind="ExternalInput")
    # aux: [0:512) mask pair (B0|B1), [512:640) identity, [640:768) ones
    aux = nc.dram_tensor("aux", [P, 768], bf16, kind="ExternalInput")
    out = nc.dram_tensor("out", [B, S, H], bf16, kind="ExternalOutput")

    with tile.TileContext(nc) as tc:
        with tc.tile_pool(name="const", bufs=1) as cpool:
            w_sb = cpool.tile([P, NHT, FPC], bf16)
            nc.sync.dma_start(w_sb[:], wqkv.rearrange("(t p) f -> p t f", p=P))
            wo_sb = cpool.tile([P, HPC, H], bf16)
            nc.sync.dma_start(wo_sb[:], wo.rearrange("(t p) o -> p t o", p=P))
            aux_sb = cpool.tile([P, 768], bf16)
            nc.sync.dma_start(aux_sb[:], aux[:])
            rot_sb = cpool.tile([P, 2 * S], bf16)
            nc.sync.dma_start(rot_sb[:, 0:S], cos_t[:])
            nc.sync.dma_start(rot_sb[:, S : 2 * S], sin_s[:])

            msk = aux_sb[:, 0:512]          # [k,128] x (B0|B1) for diag pair
            iden = aux_sb[:, 512:640]       # identity
            ones = aux_sb[:, 640:768]       # all-ones

            with (
                tc.tile_pool(name="qk", bufs=2) as qkpool,
                tc.tile_pool(name="vv", bufs=2) as vpool,
                tc.tile_pool(name="work", bufs=2) as wpool,
                tc.tile_pool(name="attn", bufs=3) as apool,
                tc.tile_pool(name="outp", bufs=3) as opool,
                tc.psum_pool(name="pproj", bufs=2) as pps,
                tc.psum_pool(name="psc", bufs=2) as scps,
                tc.psum_pool(name="pyt", bufs=2) as ytps,
                tc.psum_pool(name="pop", bufs=2) as opps,
            ):
                qk_raw = {}   # (b, f) -> raw (pre-rope) tiles
                qk_rope = {}  # (b, f) -> roped tiles
                v_sb = {}     # b -> v tiles [tok_part, ktile, hl*128]
                for b in range(B):
                    for f in range(4):
                        qk_raw[b, f] = qkpool.tile(
                            [P, S], bf16, tag=f"qkr{f}", name=f"qkr{f}_{b}"
                        )
                        qk_rope[b, f] = qkpool.tile(
                            [P, S], bf16, tag=f"qkf{f}", name=f"qkf{f}_{b}"
                        )
                    v_sb[b] = vpool.tile(
                        [P, (S // P) * 2 * P], bf16, tag="v", name=f"v_{b}"
                    )

                # ---------------- projection (+rope) for both batches -------
                for b in range(B):
                    for c in range(NCH):
                        xt_sb = wpool.tile([P, NHT, CH], bf16, tag="xt")
                        nc.sync.dma_start(
                            xt_sb[:],
                            xt[b, :, c * CH : (c + 1) * CH].rearrange(
                                "(t p) c -> p t c", p=P
                            ),
                        )
                        # q0,q1,k0,k1 : [feat, tok]
                        for f in range(4):
                            ps = pps.tile([P, CH], f32, tag="proj")
                            for h in range(NHT):
                                nc.tensor.matmul(
                                    ps[:],
                                    lhsT=w_sb[:, h, f * P : (f + 1) * P],
                                    rhs=xt_sb[:, h, :],
                                    start=(h == 0),
                                    stop=(h == NHT - 1),
                                )
                            if f % 2 == 0:
                                nc.scalar.copy(
                                    qk_raw[b, f][:, c * CH : (c + 1) * CH], ps[:]
                                )
                            else:
                                nc.vector.tensor_copy(
                                    qk_raw[b, f][:, c * CH : (c + 1) * CH], ps[:]
                                )
                        # v: [tok, feat] two tok-subs per psum tile
                        for half in range(2):
                            psv = pps.tile([P, CH], f32, tag="proj")
                            for sub in range(2):
                                tsub = half * 2 + sub
                                for h in range(NHT):
                                    nc.tensor.matmul(
                                        psv[:, sub * 2 * P : (sub + 1) * 2 * P],
                                        lhsT=xt_sb[:, h, tsub * P : (tsub + 1) * P],
                                        rhs=w_sb[:, h, 4 * P : 6 * P],
                                        start=(h == 0),
                                        stop=(h == NHT - 1),
                                    )
                            kt0 = c * 4 + half * 2
                            if half == 0:
                                nc.scalar.copy(
                                    v_sb[b][:, kt0 * 2 * P : (kt0 + 2) * 2 * P], psv[:]
                                )
                            else:
                                nc.vector.tensor_copy(
                                    v_sb[b][:, kt0 * 2 * P : (kt0 + 2) * 2 * P], psv[:]
                                )
                    # rope for the whole batch (overlaps next batch proj /
                    # previous batch attn on PE)
                    for f in range(4):
                        qsw = wpool.tile([P, S], bf16, tag="qsw")
                        nc.sync.dma_start(qsw[0:64, :], qk_raw[b, f][64:128, :])
                        nc.sync.dma_start(qsw[64:128, :], qk_raw[b, f][0:64, :])
                        m1 = wpool.tile([P, S], bf16, tag="m1")
                        nc.vector.tensor_mul(m1[:], qk_raw[b, f][:], rot_sb[:, 0:S])
                        nc.vector.tensor_mul(qsw[:], qsw[:], rot_sb[:, S : 2 * S])
                        nc.vector.tensor_add(qk_rope[b, f][:], m1[:], qsw[:])

                # ---------------- attention + out-proj ----------------------
                for b in range(B):
                    for qc in range(NQC):
                        yn = []
                        for hl in range(2):
                            q_t = qk_rope[b, hl]
                            k_t = qk_rope[b, 2 + hl]
                            # yt [:,0:256] attn*v accum; [:,256:512] denom accum
                            yts = ytps.tile([P, 2 * QC], f32, tag="yt")
                            npair = qc + 1
                            for g in range(npair):
                                sc = scps.tile([P, 2 * QC], f32, tag="sc")
                                diag = g == qc
                                for j in range(2):
                                    kb = 2 * g + j
                                    nc.tensor.matmul(
                                        sc[:, j * QC : (j + 1) * QC],
                                        lhsT=k_t[:, kb * P : (kb + 1) * P],
                                        rhs=q_t[:, qc * QC : (qc + 1) * QC],
                                        start=True,
                                        stop=not diag,
                                    )
                                    if diag:
                                        nc.tensor.matmul(
                                            sc[:, j * QC : (j + 1) * QC],
                                            lhsT=iden,
                                            rhs=msk[:, j * QC : (j + 1) * QC],
                                            start=False,
                                            stop=True,
                                        )
                                ex = apool.tile([P, 2 * QC], bf16, tag="ex")
                                nc.scalar.activation(ex[:], sc[:], AF.Exp)
                                for j in range(2):
                                    kb = 2 * g + j
                                    exj = ex[:, j * QC : (j + 1) * QC]
                                    nc.tensor.matmul(
                                        yts[:, 0:QC],
                                        lhsT=v_sb[b][
                                            :,
                                            kb * 2 * P + hl * P : kb * 2 * P + (hl + 1) * P,
                                        ],
                                        rhs=exj,
                                        start=(kb == 0),
                                        stop=(kb == 2 * qc + 1),
                                    )
                                    nc.tensor.matmul(
                                        yts[:, QC : 2 * QC],
                                        lhsT=ones,
                                        rhs=exj,
                                        start=(kb == 0),
                                        stop=(kb == 2 * qc + 1),
                                    )
                            recip = apool.tile([P, QC], f32, tag="rc")
                            nc.vector.reciprocal(recip[:], yts[:, QC : 2 * QC])
                            y = apool.tile([P, QC], bf16, tag=f"yn{hl}")
                            nc.vector.tensor_mul(y[:], yts[:, 0:QC], recip[:])
                            yn.append(y)
                        # out-proj: out[tok, ofeat], psum [128, 512] quarters
                        for sub in range(2):
                            os_sb = opool.tile([P, H], bf16, tag="os")
                            for quarter in range(4):
                                ops = opps.tile([P, 512], f32, tag="op")
                                for hl in range(2):
                                    nc.tensor.matmul(
                                        ops[:],
                                        lhsT=yn[hl][:, sub * P : (sub + 1) * P],
                                        rhs=wo_sb[:, hl, quarter * 512 : (quarter + 1) * 512],
                                        start=(hl == 0),
                                        stop=(hl == 1),
                                    )
                                if quarter % 2 == 0:
                                    nc.scalar.copy(
                                        os_sb[:, quarter * 512 : (quarter + 1) * 512],
                                        ops[:],
                                    )
                                else:
                                    nc.vector.tensor_copy(
                                        os_sb[:, quarter * 512 : (quarter + 1) * 512],
                                        ops[:],
                                    )
                            t0 = qc * QC + sub * P
                            nc.sync.dma_start(out[b, t0 : t0 + P, :], os_sb[:])
    nc.finalize()
    return nc


_NC_CACHE = None


def _get_program():
    global _NC_CACHE
    if _NC_CACHE is None:
        _NC_CACHE = _build_program()
    return _NC_CACHE


def kernel(x, rotary, qkv_weight, o_weight):
    import jax
    import ml_dtypes
    from concourse.bass_utils import run_bass_kernel_spmd

    bf = ml_dtypes.bfloat16
    cpu = jax.devices("cpu")[0]
    with jax.default_device(cpu):
        import jax.numpy as jnp

        sq = jnp.mean(jnp.abs(jnp.asarray(qkv_weight)))
        wq_q = np.asarray(jnp.round(jnp.asarray(qkv_weight) / (sq + EPS)), np.float32)
        so = jnp.mean(jnp.abs(jnp.asarray(o_weight)))
        wo_q = np.asarray(jnp.round(jnp.asarray(o_weight) / (so + EPS)), np.float32)
        sq = float(sq)
        so = float(so)

    xt = np.ascontiguousarray(x.transpose(0, 2, 1)).astype(bf)
    cos_t = np.ascontiguousarray(rotary[1].T).astype(bf)
    sin_t = np.ascontiguousarray(rotary[0].T).astype(np.float32)
    sin_s = sin_t.copy()
    sin_s[:64] *= -1.0
    sin_s = sin_s.astype(bf)

    # aux: mask pair for the diagonal k-tile pair, identity, ones
    kk = np.arange(P)[:, None]
    qq = np.arange(QC)[None, :]
    aux = np.zeros((P, 768), np.float32)
    aux[:, 0:QC] = np.where(qq < kk, MASKV, 0.0)          # B0: k-tile 2qc
    aux[:, QC : 2 * QC] = np.where(qq < kk + P, MASKV, 0.0)  # B1: k-tile 2qc+1
    aux[:, 512:640] = np.eye(P)
    aux[:, 640:768] = 1.0
    aux = aux.astype(bf)

    sm_scale = np.float32(sq * sq / math.sqrt(HEAD_DIM))
    final_scale = sq * so

    in_maps = []
    for c in range(NCORES):
        rows = []
        for part in range(3):  # q, k, v blocks of qkv_weight
            for hl in range(HPC):
                g = HPC * c + hl
                blk = wq_q[part * H + g * HEAD_DIM : part * H + (g + 1) * HEAD_DIM]
                if part == 0:
                    blk = blk * sm_scale
                rows.append(blk)
        wqkv_c = np.ascontiguousarray(np.concatenate(rows, axis=0).T).astype(bf)
        wo_c = np.ascontiguousarray(
            (wo_q[:, c * FPC // 3 : (c + 1) * FPC // 3].T * final_scale)
        ).astype(bf)
        in_maps.append(
            {
                "xt": xt,
                "wqkv": wqkv_c,
                "wo": wo_c,
                "cos_t": cos_t,
                "sin_s": sin_s,
                "aux": aux,
            }
        )

    nc = _get_program()
    res = run_bass_kernel_spmd(nc, in_maps, core_ids=list(range(NCORES)))
    acc = res.results[0]["out"].astype(np.float32)
    for c in range(1, NCORES):
        acc = acc + res.results[c]["out"].astype(np.float32)
    return acc


# revision 19
# speedup vs baseline: 1.3746x; 1.0338x over previous
"""Megatron-style tensor-parallel causal attention (BitLinear qkv/o) on 8 TRN2 cores.

Sharding: each core owns 2 of 16 heads (qkv_weight rows) and the matching
256 o_weight columns. x/rotary replicated; partial outputs summed on host.

All SBUF data is f16 (halves DMA + enables DVE 2x modes); PSUM stays fp32.
Causal mask is folded into the score PSUM via an identity-lhsT matmul adding
-60 to masked entries before exp. Softmax denominator uses the all-ones
lhsT matmul (broadcast rows), normalization on DVE before the out-proj.
Emission order proj(b0), proj(b1), attn(b0), attn(b1) keeps the PE busy:
RoPE(b0) on DVE overlaps proj(b1) matmuls, attention overlaps nothing it
needs.
"""

import math

import numpy as np

EPS = 1e-5
NUM_HEADS = 16
HEAD_DIM = 128
B, S, H = 2, 2048, 2048
NCORES = 8
HPC = NUM_HEADS // NCORES        # heads per core = 2
FPC = 3 * HPC * HEAD_DIM         # qkv features per core = 768
P = 128
NHT = H // P                     # 16 h_in tiles
CH = 512                         # proj token chunk
NCH = S // CH                    # 4 chunks per batch
QC = 256                         # attention q chunk
NQC = S // QC                    # 8
MASKV = -60.0


def _build_program():
    import concourse.bacc as bacc
    import concourse.mybir as mybir
    import concourse.tile as tile

    f32 = mybir.dt.float32
    f16 = mybir.dt.float16
    AF = mybir.ActivationFunctionType

    nc = bacc.Bacc(None, target_bir_lowering=False)

    xt = nc.dram_tensor("xt", [B, H, S], f16, kind="ExternalInput")
    wqkv = nc.dram_tensor("wqkv", [H, FPC], f16, kind="ExternalInput")
    wo = nc.dram_tensor("wo", [HPC * HEAD_DIM, H], f16, kind="ExternalInput")
    cos_t = nc.dram_tensor("cos_t", [P, S], f16, kind="ExternalInput")
    sin_s = nc.dram_tensor("sin_s", [P, S], f16, kind="ExternalInput")
    # aux: [0:512) mask pair (B0|B1), [512:640) identity, [640:768) ones
    aux = nc.dram_tensor("aux", [P, 832], f16, kind="ExternalInput")
    out = nc.dram_tensor("out", [B, S, H], f16, kind="ExternalOutput")

    with tile.TileContext(nc) as tc:
        with tc.tile_pool(name="const", bufs=1) as cpool:
            # first proj chunk's x and the first weight slice lead the DMA
            # queue so the PE starts ~9us in instead of ~24us.
            w_sb = cpool.tile([P, NHT, FPC], f16)
            wre = wqkv.rearrange("(t p) f -> p t f", p=P)
            nc.sync.dma_start(w_sb[:, 0:4, :], wre[:, 0:4, :])

            with (
                tc.tile_pool(name="qk", bufs=2) as qkpool,
                tc.tile_pool(name="vv", bufs=2) as vpool,
                tc.tile_pool(name="work", bufs=2) as wpool,
                tc.tile_pool(name="attn", bufs=3) as apool,
                tc.tile_pool(name="outp", bufs=3) as opool,
                tc.psum_pool(name="pproj", bufs=2) as pps,
                tc.psum_pool(name="psc", bufs=2) as scps,
                tc.psum_pool(name="pyt", bufs=2) as ytps,
                tc.psum_pool(name="pop", bufs=2) as opps,
            ):
                xt0 = wpool.tile([P, NHT, CH], f16, tag="xt")
                nc.sync.dma_start(
                    xt0[:], xt[0, :, 0:CH].rearrange("(t p) c -> p t c", p=P)
                )
                for hgrp in range(1, 4):
                    nc.sync.dma_start(
                        w_sb[:, 4 * hgrp : 4 * (hgrp + 1), :],
                        wre[:, 4 * hgrp : 4 * (hgrp + 1), :],
                    )
                wo_sb = cpool.tile([P, HPC, H], f16)
                nc.sync.dma_start(wo_sb[:], wo.rearrange("(t p) o -> p t o", p=P))
                aux_sb = cpool.tile([P, 832], f16)
                nc.sync.dma_start(aux_sb[:], aux[:])
                rot_sb = cpool.tile([P, 2 * S], f16)
                nc.sync.dma_start(rot_sb[:, 0:S], cos_t[:])
                nc.sync.dma_start(rot_sb[:, S : 2 * S], sin_s[:])

                msk = aux_sb[:, 0:512]          # [k,128] x (B0|B1) for diag pair
                iden = aux_sb[:, 512:640]       # identity
                ones = aux_sb[:, 640:768]       # all-ones
                expb = aux_sb[:, 768:769]       # exp bias column (-8)

                qk_raw = {}   # (b, f) -> raw (pre-rope) tiles
                qk_rope = {}  # (b, f) -> roped tiles
                v_sb = {}     # b -> v tiles [tok_part, ktile, hl*128]
                for b in range(B):
                    for f in range(4):
                        qk_raw[b, f] = qkpool.tile(
                            [P, S], f16, tag=f"qkr{f}", name=f"qkr{f}_{b}"
                        )
                        qk_rope[b, f] = qkpool.tile(
                            [P, S], f16, tag=f"qkf{f}", name=f"qkf{f}_{b}"
                        )
                    v_sb[b] = vpool.tile(
                        [P, (S // P) * 2 * P], f16, tag="v", name=f"v_{b}"
                    )

                # ---------------- projection (+rope) for both batches -------
                for b in range(B):
                    for c in range(NCH):
                        if b == 0 and c == 0:
                            xt_sb = xt0
                        else:
                            xt_sb = wpool.tile([P, NHT, CH], f16, tag="xt")
                            nc.sync.dma_start(
                                xt_sb[:],
                                xt[b, :, c * CH : (c + 1) * CH].rearrange(
                                    "(t p) c -> p t c", p=P
                                ),
                            )
                        # q0,q1,k0,k1 : [feat, tok]
                        for f in range(4):
                            ps = pps.tile([P, CH], f32, tag="proj")
                            for h in range(NHT):
                                nc.tensor.matmul(
                                    ps[:],
                                    lhsT=w_sb[:, h, f * P : (f + 1) * P],
                                    rhs=xt_sb[:, h, :],
                                    start=(h == 0),
                                    stop=(h == NHT - 1),
                                )
                            if f % 2 == 0:
                                nc.scalar.copy(
                                    qk_raw[b, f][:, c * CH : (c + 1) * CH], ps[:]
                                )
                            else:
                                nc.vector.tensor_copy(
                                    qk_raw[b, f][:, c * CH : (c + 1) * CH], ps[:]
                                )
                        # v: [tok, feat] two tok-subs per psum tile
                        for half in range(2):
                            psv = pps.tile([P, CH], f32, tag="proj")
                            for sub in range(2):
                                tsub = half * 2 + sub
                                for h in range(NHT):
                                    nc.tensor.matmul(
                                        psv[:, sub * 2 * P : (sub + 1) * 2 * P],
                                        lhsT=xt_sb[:, h, tsub * P : (tsub + 1) * P],
                                        rhs=w_sb[:, h, 4 * P : 6 * P],
                                        start=(h == 0),
                                        stop=(h == NHT - 1),
                                    )
                            kt0 = c * 4 + half * 2
                            if half == 0:
                                nc.scalar.copy(
                                    v_sb[b][:, kt0 * 2 * P : (kt0 + 2) * 2 * P], psv[:]
                                )
                            else:
                                nc.vector.tensor_copy(
                                    v_sb[b][:, kt0 * 2 * P : (kt0 + 2) * 2 * P], psv[:]
                                )
                    # rope for the whole batch (overlaps next batch proj /
                    # previous batch attn on PE)
                    for f in range(4):
                        qsw = wpool.tile([P, S], f16, tag="qsw")
                        nc.sync.dma_start(qsw[0:64, :], qk_raw[b, f][64:128, :])
                        nc.sync.dma_start(qsw[64:128, :], qk_raw[b, f][0:64, :])
                        m1 = wpool.tile([P, S], f16, tag="m1")
                        nc.vector.tensor_mul(m1[:], qk_raw[b, f][:], rot_sb[:, 0:S])
                        nc.vector.tensor_mul(qsw[:], qsw[:], rot_sb[:, S : 2 * S])
                        nc.vector.tensor_add(qk_rope[b, f][:], m1[:], qsw[:])

                # ---------------- attention + out-proj ----------------------
                for b in range(B):
                    for qc in range(NQC):
                        yn = []
                        for hl in range(2):
                            q_t = qk_rope[b, hl]
                            k_t = qk_rope[b, 2 + hl]
                            # separate PSUM banks: one accumulation group per
                            # 2KB zero region (attn*v vs denominator)
                            yt = ytps.tile([P, QC], f32, tag="yt", bufs=1)
                            sm = ytps.tile([P, QC], f32, tag="sum", bufs=1)
                            npair = qc + 1
                            for g in range(npair):
                                sc = scps.tile([P, 2 * QC], f32, tag="sc")
                                diag = g == qc
                                for j in range(2):
                                    kb = 2 * g + j
                                    nc.tensor.matmul(
                                        sc[:, j * QC : (j + 1) * QC],
                                        lhsT=k_t[:, kb * P : (kb + 1) * P],
                                        rhs=q_t[:, qc * QC : (qc + 1) * QC],
                                        start=True,
                                        stop=not diag,
                                    )
                                    if diag:
                                        nc.tensor.matmul(
                                            sc[:, j * QC : (j + 1) * QC],
                                            lhsT=iden,
                                            rhs=msk[:, j * QC : (j + 1) * QC],
                                            start=False,
                                            stop=True,
                                        )
                                # bias keeps exp in fp16 range; cancels in y/denom
                                ex = apool.tile([P, 2 * QC], f16, tag="ex")
                                nc.scalar.activation(ex[:], sc[:], AF.Exp, bias=expb)
                                for j in range(2):
                                    kb = 2 * g + j
                                    exj = ex[:, j * QC : (j + 1) * QC]
                                    nc.tensor.matmul(
                                        yt[:],
                                        lhsT=v_sb[b][
                                            :,
                                            kb * 2 * P + hl * P : kb * 2 * P + (hl + 1) * P,
                                        ],
                                        rhs=exj,
                                        start=(kb == 0),
                                        stop=(kb == 2 * qc + 1),
                                    )
                                    nc.tensor.matmul(
                                        sm[:],
                                        lhsT=ones,
                                        rhs=exj,
                                        start=(kb == 0),
                                        stop=(kb == 2 * qc + 1),
                                    )
                            recip = apool.tile([P, QC], f32, tag="rc")
                            nc.vector.reciprocal(recip[:], sm[:])
                            y = apool.tile([P, QC], f16, tag=f"yn{hl}")
                            nc.vector.tensor_mul(y[:], yt[:], recip[:])
                            yn.append(y)
                        # out-proj: out[tok, ofeat], psum [128, 512] quarters
                        for sub in range(2):
                            os_sb = opool.tile([P, H], f16, tag="os")
                            for quarter in range(4):
                                ops = opps.tile([P, 512], f32, tag="op")
                                for hl in range(2):
                                    nc.tensor.matmul(
                                        ops[:],
                                        lhsT=yn[hl][:, sub * P : (sub + 1) * P],
                                        rhs=wo_sb[:, hl, quarter * 512 : (quarter + 1) * 512],
                                        start=(hl == 0),
                                        stop=(hl == 1),
                                    )
                                if quarter % 2 == 0:
                                    nc.scalar.copy(
                                        os_sb[:, quarter * 512 : (quarter + 1) * 512],
                                        ops[:],
                                    )
                                else:
                                    nc.vector.tensor_copy(
                                        os_sb[:, quarter * 512 : (quarter + 1) * 512],
                                        ops[:],
                                    )
                            t0 = qc * QC + sub * P
                            nc.sync.dma_start(out[b, t0 : t0 + P, :], os_sb[:])
    nc.finalize()
    return nc


_NC_CACHE = None


def _get_program():
    global _NC_CACHE
    if _NC_CACHE is None:
        _NC_CACHE = _build_program()
    return _NC_CACHE


def _prep_in_maps(x, rotary, qkv_weight, o_weight):
    import jax
    import ml_dtypes

    bf = np.float16
    cpu = jax.devices("cpu")[0]
    with jax.default_device(cpu):
        import jax.numpy as jnp

        sq = jnp.mean(jnp.abs(jnp.asarray(qkv_weight)))
        wq_q = np.asarray(jnp.round(jnp.asarray(qkv_weight) / (sq + EPS)), np.float32)
        so = jnp.mean(jnp.abs(jnp.asarray(o_weight)))
        wo_q = np.asarray(jnp.round(jnp.asarray(o_weight) / (so + EPS)), np.float32)
        sq = float(sq)
        so = float(so)

    xt = np.ascontiguousarray(x.transpose(0, 2, 1)).astype(bf)
    cos_t = np.ascontiguousarray(rotary[1].T).astype(bf)
    sin_t = np.ascontiguousarray(rotary[0].T).astype(np.float32)
    sin_s = sin_t.copy()
    sin_s[:64] *= -1.0
    sin_s = sin_s.astype(bf)

    # aux: mask pair for the diagonal k-tile pair, identity, ones
    kk = np.arange(P)[:, None]
    qq = np.arange(QC)[None, :]
    aux = np.zeros((P, 832), np.float32)
    aux[:, 0:QC] = np.where(qq < kk, MASKV, 0.0)          # B0: k-tile 2qc
    aux[:, QC : 2 * QC] = np.where(qq < kk + P, MASKV, 0.0)  # B1: k-tile 2qc+1
    aux[:, 512:640] = np.eye(P)
    aux[:, 640:768] = 1.0
    aux[:, 768] = -8.0
    aux = aux.astype(bf)

    # fp16 scaling: sqrt(sm_scale) on BOTH q and k weights (scores land fully
    # scaled in PSUM, masks are in post-scale units); final o-scale folded
    # into v weights (keeps every fp16 tensor in normal range; o_weight stays
    # exactly ternary in fp16).
    alpha = np.float32(math.sqrt(sq * sq / math.sqrt(HEAD_DIM)))
    final_scale = np.float32(sq * so)

    in_maps = []
    for c in range(NCORES):
        rows = []
        for part in range(3):  # q, k, v blocks of qkv_weight
            for hl in range(HPC):
                g = HPC * c + hl
                blk = wq_q[part * H + g * HEAD_DIM : part * H + (g + 1) * HEAD_DIM]
                if part < 2:
                    blk = blk * alpha
                else:
                    blk = blk * final_scale
                rows.append(blk)
        wqkv_c = np.ascontiguousarray(np.concatenate(rows, axis=0).T).astype(bf)
        wo_c = np.ascontiguousarray(
            wo_q[:, c * FPC // 3 : (c + 1) * FPC // 3].T
        ).astype(bf)
        in_maps.append(
            {
                "xt": xt,
                "wqkv": wqkv_c,
                "wo": wo_c,
                "cos_t": cos_t,
                "sin_s": sin_s,
                "aux": aux,
            }
        )
    return in_maps


def kernel(x, rotary, qkv_weight, o_weight):
    from concourse.bass_utils import run_bass_kernel_spmd

    in_maps = _prep_in_maps(x, rotary, qkv_weight, o_weight)
    nc = _get_program()
    res = run_bass_kernel_spmd(nc, in_maps, core_ids=list(range(NCORES)))
    acc = res.results[0]["out"].astype(np.float32)
    for c in range(1, NCORES):
        acc = acc + res.results[c]["out"].astype(np.float32)
    return acc


# revision 20
# speedup vs baseline: 1.4995x; 1.0908x over previous
"""Megatron-style tensor-parallel causal attention (BitLinear qkv/o) on 8 TRN2 cores.

Sharding: each core owns 2 of 16 heads (qkv_weight rows) and the matching
256 o_weight columns. x/rotary replicated; partial outputs summed on host.

All SBUF data is f16 (halves DMA + enables DVE 2x modes); PSUM stays fp32.
Causal mask is folded into the score PSUM via an identity-lhsT matmul adding
-60 to masked entries before exp. Softmax denominator uses the all-ones
lhsT matmul (broadcast rows), normalization on DVE before the out-proj.
Emission order proj(b0), proj(b1), attn(b0), attn(b1) keeps the PE busy:
RoPE(b0) on DVE overlaps proj(b1) matmuls, attention overlaps nothing it
needs.
"""

import math

import numpy as np

EPS = 1e-5
NUM_HEADS = 16
HEAD_DIM = 128
B, S, H = 2, 2048, 2048
NCORES = 8
HPC = NUM_HEADS // NCORES        # heads per core = 2
FPC = 3 * HPC * HEAD_DIM         # qkv features per core = 768
P = 128
NHT = H // P                     # 16 h_in tiles
CH = 512                         # proj token chunk
NCH = S // CH                    # 4 chunks per batch
QC = 256                         # attention q chunk
NQC = S // QC                    # 8
MASKV = -60.0


def _build_program():
    import concourse.bacc as bacc
    import concourse.mybir as mybir
    import concourse.tile as tile

    f32 = mybir.dt.float32
    f16 = mybir.dt.float16
    AF = mybir.ActivationFunctionType

    nc = bacc.Bacc(None, target_bir_lowering=False)

    xt = nc.dram_tensor("xt", [B, H, S], f16, kind="ExternalInput")
    wqkv = nc.dram_tensor("wqkv", [H, FPC], f16, kind="ExternalInput")
    wo = nc.dram_tensor("wo", [HPC * HEAD_DIM, H], f16, kind="ExternalInput")
    cos_t = nc.dram_tensor("cos_t", [P, S], f16, kind="ExternalInput")
    sin_s = nc.dram_tensor("sin_s", [P, S], f16, kind="ExternalInput")
    # aux: [0:512) mask pair (B0|B1), [512:640) identity, [640:768) ones
    aux = nc.dram_tensor("aux", [P, 832], f16, kind="ExternalInput")
    out = nc.dram_tensor("out", [B, S, H], f16, kind="ExternalOutput")

    with tile.TileContext(nc) as tc:
        with tc.tile_pool(name="const", bufs=1) as cpool:
            # first proj chunk's x and the first weight slice lead the DMA
            # queue so the PE starts ~9us in instead of ~24us.
            w_sb = cpool.tile([P, NHT, FPC], f16)
            wre = wqkv.rearrange("(t p) f -> p t f", p=P)
            nc.sync.dma_start(w_sb[:, 0:4, :], wre[:, 0:4, :])

            with (
                tc.tile_pool(name="qk", bufs=2) as qkpool,
                tc.tile_pool(name="vv", bufs=2) as vpool,
                tc.tile_pool(name="work", bufs=2) as wpool,
                tc.tile_pool(name="attn", bufs=3) as apool,
                tc.tile_pool(name="outp", bufs=3) as opool,
                tc.psum_pool(name="pproj", bufs=2) as pps,
                tc.psum_pool(name="psc", bufs=2) as scps,
                tc.psum_pool(name="pyt", bufs=2) as ytps,
                tc.psum_pool(name="pop", bufs=2) as opps,
            ):
                xt0 = wpool.tile([P, NHT, CH], f16, tag="xt")
                nc.sync.dma_start(
                    xt0[:], xt[0, :, 0:CH].rearrange("(t p) c -> p t c", p=P)
                )
                for hgrp in range(1, 4):
                    nc.sync.dma_start(
                        w_sb[:, 4 * hgrp : 4 * (hgrp + 1), :],
                        wre[:, 4 * hgrp : 4 * (hgrp + 1), :],
                    )
                wo_sb = cpool.tile([P, HPC, H], f16)
                nc.sync.dma_start(wo_sb[:], wo.rearrange("(t p) o -> p t o", p=P))
                aux_sb = cpool.tile([P, 832], f16)
                nc.sync.dma_start(aux_sb[:], aux[:])
                rot_sb = cpool.tile([P, 2 * S], f16)
                nc.sync.dma_start(rot_sb[:, 0:S], cos_t[:])
                nc.sync.dma_start(rot_sb[:, S : 2 * S], sin_s[:])

                msk = aux_sb[:, 0:512]          # [k,128] x (B0|B1) for diag pair
                iden = aux_sb[:, 512:640]       # identity
                ones = aux_sb[:, 640:768]       # all-ones
                expb = aux_sb[:, 768:769]       # exp bias column (-8)

                qk_raw = {}   # (b, f) -> raw (pre-rope) tiles
                qk_rope = {}  # (b, f) -> roped tiles
                v_sb = {}     # b -> v tiles [tok_part, ktile, hl*128]
                for b in range(B):
                    for f in range(4):
                        qk_raw[b, f] = qkpool.tile(
                            [P, S], f16, tag=f"qkr{f}", name=f"qkr{f}_{b}"
                        )
                        qk_rope[b, f] = qkpool.tile(
                            [P, S], f16, tag=f"qkf{f}", name=f"qkf{f}_{b}"
                        )
                    v_sb[b] = vpool.tile(
                        [P, (S // P) * 2 * P], f16, tag="v", name=f"v_{b}"
                    )

                # ---------------- projection (+rope) for both batches -------
                for b in range(B):
                    for c in range(NCH):
                        if b == 0 and c == 0:
                            xt_sb = xt0
                        else:
                            xt_sb = wpool.tile([P, NHT, CH], f16, tag="xt")
                            nc.sync.dma_start(
                                xt_sb[:],
                                xt[b, :, c * CH : (c + 1) * CH].rearrange(
                                    "(t p) c -> p t c", p=P
                                ),
                            )
                        # q0,q1,k0,k1 : [feat, tok]
                        for f in range(4):
                            ps = pps.tile([P, CH], f32, tag="proj")
                            for h in range(NHT):
                                nc.tensor.matmul(
                                    ps[:],
                                    lhsT=w_sb[:, h, f * P : (f + 1) * P],
                                    rhs=xt_sb[:, h, :],
                                    start=(h == 0),
                                    stop=(h == NHT - 1),
                                )
                            if f % 2 == 0:
                                nc.scalar.copy(
                                    qk_raw[b, f][:, c * CH : (c + 1) * CH], ps[:]
                                )
                            else:
                                nc.vector.tensor_copy(
                                    qk_raw[b, f][:, c * CH : (c + 1) * CH], ps[:]
                                )
                        # v: [tok, feat] two tok-subs per psum tile
                        for half in range(2):
                            psv = pps.tile([P, CH], f32, tag="proj")
                            for sub in range(2):
                                tsub = half * 2 + sub
                                for h in range(NHT):
                                    nc.tensor.matmul(
                                        psv[:, sub * 2 * P : (sub + 1) * 2 * P],
                                        lhsT=xt_sb[:, h, tsub * P : (tsub + 1) * P],
                                        rhs=w_sb[:, h, 4 * P : 6 * P],
                                        start=(h == 0),
                                        stop=(h == NHT - 1),
                                    )
                            kt0 = c * 4 + half * 2
                            if half == 0:
                                nc.scalar.copy(
                                    v_sb[b][:, kt0 * 2 * P : (kt0 + 2) * 2 * P], psv[:]
                                )
                            else:
                                nc.vector.tensor_copy(
                                    v_sb[b][:, kt0 * 2 * P : (kt0 + 2) * 2 * P], psv[:]
                                )
                    # rope for the whole batch (overlaps next batch proj /
                    # previous batch attn on PE)
                    for f in range(4):
                        qsw = wpool.tile([P, S], f16, tag="qsw")
                        nc.sync.dma_start(qsw[0:64, :], qk_raw[b, f][64:128, :])
                        nc.sync.dma_start(qsw[64:128, :], qk_raw[b, f][0:64, :])
                        m1 = wpool.tile([P, S], f16, tag="m1")
                        nc.vector.tensor_mul(m1[:], qk_raw[b, f][:], rot_sb[:, 0:S])
                        nc.vector.tensor_mul(qsw[:], qsw[:], rot_sb[:, S : 2 * S])
                        nc.vector.tensor_add(qk_rope[b, f][:], m1[:], qsw[:])

                # ---------------- attention + out-proj ----------------------
                # The last k-tile of each q-chunk only covers q[128:256)
                # (ragged trim). Denominator: full pairs are pre-summed on DVE
                # (halves the ones-matmul rows); the ones-matmul for pair g is
                # deferred until after pair g+1's attn*v so the PE never waits
                # on the DVE add.
                def attn_unit(b, qc, hl):
                    q_t = qk_rope[b, hl]
                    k_t = qk_rope[b, 2 + hl]
                    qs = q_t[:, qc * QC : (qc + 1) * QC]
                    qs_hi = q_t[:, qc * QC + P : (qc + 1) * QC]
                    yt = ytps.tile([P, QC], f32, tag="yt", bufs=1)
                    sm = ytps.tile([P, QC], f32, tag="sum", bufs=1)
                    pend = None       # deferred exs tile for the ones-matmul
                    sum_started = False

                    def ones_mm(rhs_ap, region, stop):
                        nonlocal sum_started
                        nc.tensor.matmul(
                            sm[:, region[0] : region[1]],
                            lhsT=ones,
                            rhs=rhs_ap,
                            start=not sum_started,
                            stop=stop,
                        )
                        sum_started = True

                    for g in range(qc + 1):
                        diag = g == qc
                        scw = 2 * QC if not diag else QC + P
                        sc = scps.tile([P, 2 * QC], f32, tag="sc")
                        # scores (+mask on the diagonal pair)
                        nc.tensor.matmul(
                            sc[:, 0:QC],
                            lhsT=k_t[:, 2 * g * P : (2 * g + 1) * P],
                            rhs=qs,
                            start=True,
                            stop=not diag,
                        )
                        if diag:
                            nc.tensor.matmul(
                                sc[:, 0:QC], lhsT=iden, rhs=msk[:, 0:QC],
                                start=False, stop=True,
                            )
                            nc.tensor.matmul(
                                sc[:, QC : QC + P],
                                lhsT=k_t[:, (2 * g + 1) * P : (2 * g + 2) * P],
                                rhs=qs_hi,
                                start=True,
                                stop=False,
                            )
                            nc.tensor.matmul(
                                sc[:, QC : QC + P], lhsT=iden, rhs=msk[:, 0:P],
                                start=False, stop=True,
                            )
                        else:
                            nc.tensor.matmul(
                                sc[:, QC : 2 * QC],
                                lhsT=k_t[:, (2 * g + 1) * P : (2 * g + 2) * P],
                                rhs=qs,
                                start=True,
                                stop=True,
                            )
                        # exp (bias keeps fp16 range; cancels in y/denom)
                        ex = apool.tile([P, scw], f16, tag="ex")
                        nc.scalar.activation(ex[:], sc[:, 0:scw], AF.Exp, bias=expb)
                        # attn*v
                        v0 = 2 * g * 2 * P + hl * P
                        nc.tensor.matmul(
                            yt[:],
                            lhsT=v_sb[b][:, v0 : v0 + P],
                            rhs=ex[:, 0:QC],
                            start=(g == 0),
                            stop=False,
                        )
                        v1 = (2 * g + 1) * 2 * P + hl * P
                        nc.tensor.matmul(
                            yt[:, P:QC] if diag else yt[:],
                            lhsT=v_sb[b][:, v1 : v1 + P],
                            rhs=ex[:, QC:scw],
                            start=False,
                            stop=diag,
                        )
                        # deferred denominator matmul for the previous pair
                        if pend is not None:
                            ones_mm(pend[:], (0, QC), stop=False)
                            pend = None
                        if not diag:
                            exs = apool.tile([P, QC], f16, tag="exs")
                            nc.vector.tensor_add(
                                exs[:], ex[:, 0:QC], ex[:, QC : 2 * QC]
                            )
                            pend = exs
                        else:
                            ones_mm(ex[:, 0:QC], (0, QC), stop=False)
                            ones_mm(ex[:, QC : QC + P], (P, QC), stop=True)
                    recip = apool.tile([P, QC], f32, tag="rc")
                    nc.vector.reciprocal(recip[:], sm[:])
                    y = apool.tile([P, QC], f16, tag=f"yn{hl}")
                    nc.vector.tensor_mul(y[:], yt[:], recip[:])
                    return y

                def oproj(b, qc, yn):
                    for sub in range(2):
                        os_sb = opool.tile([P, H], f16, tag="os")
                        for quarter in range(4):
                            ops = opps.tile([P, 512], f32, tag="op")
                            for hl in range(2):
                                nc.tensor.matmul(
                                    ops[:],
                                    lhsT=yn[hl][:, sub * P : (sub + 1) * P],
                                    rhs=wo_sb[:, hl, quarter * 512 : (quarter + 1) * 512],
                                    start=(hl == 0),
                                    stop=(hl == 1),
                                )
                            if quarter % 2 == 0:
                                nc.scalar.copy(
                                    os_sb[:, quarter * 512 : (quarter + 1) * 512],
                                    ops[:],
                                )
                            else:
                                nc.vector.tensor_copy(
                                    os_sb[:, quarter * 512 : (quarter + 1) * 512],
                                    ops[:],
                                )
                        t0 = qc * QC + sub * P
                        nc.sync.dma_start(out[b, t0 : t0 + P, :], os_sb[:])

                pending = None
                for b in range(B):
                    for qc in range(NQC):
                        y0 = attn_unit(b, qc, 0)
                        # previous chunk's out-proj lands between the two
                        # head-units: PE work that hides the normalize chain
                        if pending is not None:
                            oproj(*pending)
                        y1 = attn_unit(b, qc, 1)
                        pending = (b, qc, [y0, y1])
                oproj(*pending)
    nc.finalize()
    return nc


_NC_CACHE = None


def _get_program():
    global _NC_CACHE
    if _NC_CACHE is None:
        _NC_CACHE = _build_program()
    return _NC_CACHE


def _prep_in_maps(x, rotary, qkv_weight, o_weight):
    import jax
    import ml_dtypes

    bf = np.float16
    cpu = jax.devices("cpu")[0]
    with jax.default_device(cpu):
        import jax.numpy as jnp

        sq = jnp.mean(jnp.abs(jnp.asarray(qkv_weight)))
        wq_q = np.asarray(jnp.round(jnp.asarray(qkv_weight) / (sq + EPS)), np.float32)
        so = jnp.mean(jnp.abs(jnp.asarray(o_weight)))
        wo_q = np.asarray(jnp.round(jnp.asarray(o_weight) / (so + EPS)), np.float32)
        sq = float(sq)
        so = float(so)

    xt = np.ascontiguousarray(x.transpose(0, 2, 1)).astype(bf)
    cos_t = np.ascontiguousarray(rotary[1].T).astype(bf)
    sin_t = np.ascontiguousarray(rotary[0].T).astype(np.float32)
    sin_s = sin_t.copy()
    sin_s[:64] *= -1.0
    sin_s = sin_s.astype(bf)

    # aux: mask pair for the diagonal k-tile pair, identity, ones
    kk = np.arange(P)[:, None]
    qq = np.arange(QC)[None, :]
    aux = np.zeros((P, 832), np.float32)
    aux[:, 0:QC] = np.where(qq < kk, MASKV, 0.0)          # B0: k-tile 2qc
    aux[:, QC : 2 * QC] = np.where(qq < kk + P, MASKV, 0.0)  # B1: k-tile 2qc+1
    aux[:, 512:640] = np.eye(P)
    aux[:, 640:768] = 1.0
    aux[:, 768] = -8.0
    aux = aux.astype(bf)

    # fp16 scaling: sqrt(sm_scale) on BOTH q and k weights (scores land fully
    # scaled in PSUM, masks are in post-scale units); final o-scale folded
    # into v weights (keeps every fp16 tensor in normal range; o_weight stays
    # exactly ternary in fp16).
    alpha = np.float32(math.sqrt(sq * sq / math.sqrt(HEAD_DIM)))
    final_scale = np.float32(sq * so)

    in_maps = []
    for c in range(NCORES):
        rows = []
        for part in range(3):  # q, k, v blocks of qkv_weight
            for hl in range(HPC):
                g = HPC * c + hl
                blk = wq_q[part * H + g * HEAD_DIM : part * H + (g + 1) * HEAD_DIM]
                if part < 2:
                    blk = blk * alpha
                else:
                    blk = blk * final_scale
                rows.append(blk)
        wqkv_c = np.ascontiguousarray(np.concatenate(rows, axis=0).T).astype(bf)
        wo_c = np.ascontiguousarray(
            wo_q[:, c * FPC // 3 : (c + 1) * FPC // 3].T
        ).astype(bf)
        in_maps.append(
            {
                "xt": xt,
                "wqkv": wqkv_c,
                "wo": wo_c,
                "cos_t": cos_t,
                "sin_s": sin_s,
                "aux": aux,
            }
        )
    return in_maps


def kernel(x, rotary, qkv_weight, o_weight):
    from concourse.bass_utils import run_bass_kernel_spmd

    in_maps = _prep_in_maps(x, rotary, qkv_weight, o_weight)
    nc = _get_program()
    res = run_bass_kernel_spmd(nc, in_maps, core_ids=list(range(NCORES)))
    acc = res.results[0]["out"].astype(np.float32)
    for c in range(1, NCORES):
        acc = acc + res.results[c]["out"].astype(np.float32)
    return acc
